# revision 24
# baseline (speedup 1.0000x reference)
"""De-stationary causal attention (B=2, L=S=2048, H=8, E=64) on 8 TRN2 cores.

Sharding: the 16 (batch, head) pairs are distributed 2-per-core (cores 0-3
get batch 0, heads 0..7; cores 4-7 get batch 1). Each core runs the same
Bass program on its two pairs.

Math: logits = (Q K^T) * (tau/sqrt(E)) + delta/sqrt(E), causal softmax, A V.
Host-side folds: Q is pre-scaled by tau/sqrt(E); exp(delta/sqrt(E)) is folded
into V (and into the appended denominator column), because
softmax(x + d)_s = exp(x_s) e^{d_s} / sum_j exp(x_j) e^{d_j}.
So the device only computes exp(q'k) with no bias.

Device structure per (b,h) pair, scores kept TRANSPOSED (s on partitions):
  bank-major over 4 output l-blocks of 512; for each bank, groups of 4
  s-tiles: ST[s,l] row-packed on the PE (two k=64 matmuls on partition halves
  run concurrently), exp on the ACT engine (diagonal groups repack their
  valid column ranges contiguously so 2 ACT calls cover a group instead of
  4), causal-mask multiply on the DVE, then AV row-packed into a [65,512]
  accumulator whose row 64 is the softmax denominator. The accumulator is
  copied to SBUF and DMA'd out RAW as [65, L] — the host does the
  denominator division and the [E,L] -> [L,E] transpose in numpy, which
  removes all PE transposes and DVE normalization from the device.

DMA: V arrives pre-tiled [128, NT, 66] and Q/K as whole [64, L] slabs
(one descriptor set each); the duplicate partition-half is an SBUF->SBUF
DMA on the GpSimd queue.
"""

import copy
import sys

import numpy as np

try:
    import concourse.bass as bass
except ImportError:  # pragma: no cover
    sys.path.insert(0, "/opt/trn_rl_repo")
    import concourse.bass as bass

import concourse.mybir as mybir
import concourse.tile as tile
from concourse.bass_utils import run_bass_kernel_spmd
from concourse.vector_clock import ScopedClock

B, L, H, E = 2, 2048, 8, 64
N_CORES = 8
PAIRS_PER_CORE = 2
SCALE = 1.0 / np.sqrt(np.float32(E))  # 0.125

f32 = mybir.dt.float32
f32r = mybir.dt.float32r
bf16 = mybir.dt.bfloat16

# ---------------------------------------------------------------------------
# Walrus in this toolchain rejects >1 sync-wait per instruction. Split extra
# waits onto NoOps committed just before the instruction on the same engine.
# ---------------------------------------------------------------------------
_NOP_TEMPLATE = {}


def _make_nop(engine, name):
    if engine not in _NOP_TEMPLATE:
        tmp = bass.Bass()
        _NOP_TEMPLATE[engine] = tmp.engines[engine].nop(nofuse=True).ins
    nop = copy.copy(_NOP_TEMPLATE[engine])
    nop.name = name
    nop.engine = engine
    nop.sync_info = None
    return nop


class SplitWaitTileContext(tile.TileContext):
    _ws_counter = 0

    def _split_waits(self, inst):
        si = inst.sync_info
        if si is None or not si.on_wait or len(si.on_wait) <= 1:
            return []
        if inst.engine == mybir.EngineType.Unassigned:
            return []
        waits = list(si.on_wait)
        inst.sync_info = mybir.SyncInfo(
            on_wait=[waits[0]], on_update=list(si.on_update or [])
        )
        nops = []
        for w in waits[1:]:
            SplitWaitTileContext._ws_counter += 1
            nop = _make_nop(inst.engine, f"I-ws{SplitWaitTileContext._ws_counter}")
            nop.sync_info = mybir.SyncInfo(on_wait=[w], on_update=[])
            nops.append(nop)
        return nops

    def _commit_instruction(self, inst, lazy_reg_writes=True):
        for nop in self._split_waits(inst):
            self._add_instruction(nop)
        super()._commit_instruction(inst, lazy_reg_writes)

    def _drain_and_barrier(self, tick_clock, wait_clock):
        nc = self.nc
        probe = nc.sync.nop(nofuse=True)
        wait_clock.add_sem_waits(
            probe.ins, ScopedClock({None: tick_clock.global_clock})
        )
        waits = list(probe.ins.sync_info.on_wait or []) if probe.ins.sync_info else []
        if len(waits) > 1:
            probe.ins.sync_info.on_wait = [waits[0]]
            handles = {h.num: h for h in self.sems.allocated().values()}
            for w in waits[1:]:
                nop = nc.sync.nop(nofuse=True)
                nop.wait_op(handles[w.id], w.wait_value, "sem-ge")
        nc.sync.drain()

        nc.all_engine_barrier()
        assert self.sems is not None
        popped = nc._tile_sem_poison_stack.pop()
        assert popped is self._sem_poison
        nc.clear_and_free_semaphores(list(self.sems.allocated().values()))
        nc.all_engine_barrier()


# ---------------------------------------------------------------------------
# Program builder (bank-major, fully row-packed)
# ---------------------------------------------------------------------------

# Diagonal-group column plan: chunk c covers local l-cols [128c, 512) of the
# bank (width 512-128c). The valid ranges are packed contiguously: tile0
# holds c0|c1|c3 at columns 0/512/896, tile1 holds c2 at column 0 (a_grp
# column 1024). One ACT call per tile instead of one per chunk.
DIAG_BASE = {0: 0, 1: 512, 2: 1024, 3: 896}


def build_program(st_dtype=bf16, av_dtype=bf16):
    nc = bass.Bass()
    Exp = mybir.ActivationFunctionType.Exp

    VW = E + 2  # v row: 64 values + denominator col + pad
    NT = L // 128  # 16 s-tiles / l-tiles
    NB = L // 512  # 4 OT banks

    qt = nc.declare_dram_parameter("qt", [PAIRS_PER_CORE, E, L], st_dtype, isOutput=False)
    # K^T packed even/odd: rows 0:64 hold the even 128-col s-tile blocks,
    # rows 64:128 the odd ones (the row-packed score matmuls only ever read
    # even s-tiles from the low partition half and odd from the high half,
    # so no duplicate K transfer is needed).
    kt = nc.declare_dram_parameter(
        "kt", [PAIRS_PER_CORE, 2 * E, L // 2], st_dtype, isOutput=False
    )
    vv = nc.declare_dram_parameter(
        "vv", [PAIRS_PER_CORE, 128, NT, VW], av_dtype, isOutput=False
    )
    mask = nc.declare_dram_parameter("mask", [128, 128], av_dtype, isOutput=False)
    oo = nc.declare_dram_parameter(
        "oo", [PAIRS_PER_CORE, E + 1, L], f32, isOutput=True
    )

    with SplitWaitTileContext(nc) as tc:
        with (
            tc.tile_pool(name="const", bufs=1) as constp,
            tc.tile_pool(name="qk", bufs=2) as qkp,
            tc.tile_pool(name="vp", bufs=2) as vp,
            tc.tile_pool(name="ap", bufs=4) as ap_pool,
            tc.tile_pool(name="ep", bufs=2) as ep,
            tc.tile_pool(name="st", bufs=3, space="PSUM") as stp,
            tc.tile_pool(name="otp", bufs=2, space="PSUM") as otp,
        ):
            mask_sb = constp.tile([128, 128], av_dtype, tag="mask")
            consts_loaded = []

            for pair in range(PAIRS_PER_CORE):
                # Q^T on partitions 0-63 with a copy on 64-127 (the stream
                # operand is needed on both halves); K^T arrives pre-packed
                # even/odd so one [128, L/2] transfer serves both halves.
                qt_sb = qkp.tile([2 * E, L], st_dtype, tag="qt")
                kt_sb = qkp.tile([2 * E, L // 2], st_dtype, tag="kt")
                # V slab [128, 16, 66]; col 64 carries exp(delta') for the
                # softmax denominator (host-folded), col 65 is padding.
                v_sb = vp.tile([128, NT, VW], av_dtype, tag="v")
                # pair 0 loads in small leading chunks so the first score
                # group starts as early as possible; pair 1 loads whole
                # while pair 0 computes. Queues: K + V on sync, Q low half
                # on scalar; the Q high-half duplicate is a cheap DVE
                # SBUF->SBUF copy (no extra HBM traffic).
                if pair == 0:
                    kchunks = [
                        slice(0, L // 8),
                        slice(L // 8, L // 4),
                        slice(L // 4, L // 2),
                    ]
                    qchunks = [
                        slice(0, L // 4),
                        slice(L // 4, L // 2),
                        slice(L // 2, L),
                    ]
                    vchunks = [slice(0, NT // 2), slice(NT // 2, NT)]
                else:
                    kchunks = [slice(0, L // 2)]
                    qchunks = [slice(0, L)]
                    vchunks = [slice(0, NT)]
                for cl in kchunks:
                    nc.sync.dma_start(out=kt_sb[:, cl], in_=kt[pair][:, cl])
                    if not consts_loaded:
                        # mask rides behind the first k chunk; first needed
                        # by group (0,0)'s diagonal multiply
                        nc.sync.dma_start(out=mask_sb, in_=mask[:])
                        consts_loaded.append(True)
                for tv in vchunks:
                    nc.sync.dma_start(out=v_sb[:, tv, :], in_=vv[pair][:, tv, :])
                for cl in qchunks:
                    nc.scalar.dma_start(out=qt_sb[0:E, cl], in_=qt[pair][:, cl])
                    nc.vector.tensor_copy(
                        qt_sb[E : 2 * E, cl], qt_sb[0:E, cl]
                    )

                ot_banks = {}

                def emit_st_group(lj, gi):
                    """Returns (a_grp, bases) where bases[c] = (a_grp column
                    base, l-offset off) for chunk c."""
                    a_grp = ap_pool.tile(
                        [128, 4 * 512], av_dtype, tag="A", name="A"
                    )
                    if gi != lj:
                        for hb in range(2):
                            st = stp.tile(
                                [128, 1024], f32, tag="st", name="st"
                            )
                            for cc in range(2):
                                c = 2 * hb + cc
                                si = 4 * gi + c
                                half = (c % 2) * E
                                nc.tensor.matmul(
                                    st[:, 512 * cc : 512 * (cc + 1)],
                                    kt_sb[half : half + E, (si // 2) * 128 : (si // 2) * 128 + 128],
                                    qt_sb[half : half + E, 512 * lj : 512 * lj + 512],
                                    start=True,
                                    stop=True,
                                )
                            nc.scalar.activation(
                                out=a_grp[:, 1024 * hb : 1024 * (hb + 1)],
                                in_=st,
                                func=Exp,
                                scale=1.0,
                            )
                        return a_grp, [(512 * c, 0) for c in range(4)]
                    # Diagonal group: chunk c valid over l-cols [128c, 512).
                    # tile0 <- c0|c1|c3 packed at 0/512/896, tile1 <- c2 at 0.
                    st0 = stp.tile([128, 1024], f32, tag="st", name="st")
                    st1 = stp.tile([128, 1024], f32, tag="st", name="st")
                    for c in range(4):
                        si = 4 * gi + c
                        off = 128 * c
                        half = (c % 2) * E
                        dst = st1 if c == 2 else st0
                        base = 0 if c == 2 else DIAG_BASE[c]
                        nc.tensor.matmul(
                            dst[:, base : base + 512 - off],
                            kt_sb[half : half + E, (si // 2) * 128 : (si // 2) * 128 + 128],
                            qt_sb[half : half + E, 512 * lj + off : 512 * lj + 512],
                            start=True,
                            stop=True,
                        )
                    nc.scalar.activation(
                        out=a_grp[:, 0:1024], in_=st0, func=Exp, scale=1.0
                    )
                    nc.scalar.activation(
                        out=a_grp[:, 1024:1280],
                        in_=st1[:, 0:256],
                        func=Exp,
                        scale=1.0,
                    )
                    # causal mask on the leading 128 cols of each chunk
                    # (its diagonal block), split across DVE and gpsimd so
                    # the four multiplies run two-by-two in parallel.
                    for c in range(4):
                        base = DIAG_BASE[c]
                        eng = nc.vector if c % 2 == 0 else nc.gpsimd
                        eng.tensor_mul(
                            a_grp[:, base : base + 128],
                            a_grp[:, base : base + 128],
                            mask_sb,
                        )
                    return a_grp, [
                        (DIAG_BASE[c], 128 * c) for c in range(4)
                    ]

                def emit_av_group(lj, gi, a_grp, bases):
                    ot = ot_banks[lj]
                    for c in range(4):
                        si = 4 * gi + c
                        base, off = bases[c]
                        nc.tensor.matmul(
                            ot[:, off:512],
                            v_sb[:, si, 0 : E + 1],
                            a_grp[:, base : base + 512 - off],
                            start=(gi == 0 and c == 0),
                            stop=(gi == lj and c == 3),
                        )

                def epilogue(lj, sliced=False):
                    # Copy the raw accumulator (row 64 = denominator) to
                    # SBUF and ship it out; the host divides + transposes.
                    # The very last bank is sliced in half so its store
                    # overlaps the second copy.
                    ot = ot_banks.pop(lj)
                    ot_sb = ep.tile([E + 1, 512], f32, tag="ot_sb", name="ot_sb")
                    cuts = [slice(0, 256), slice(256, 512)] if sliced else [
                        slice(0, 512)
                    ]
                    for ci, cu in enumerate(cuts):
                        nc.vector.tensor_copy(ot_sb[:, cu], ot[:, cu])
                        # sliced halves issue on different queues so their
                        # descriptor generation overlaps
                        eng = nc.scalar if (sliced and ci == 0) else nc.sync
                        eng.dma_start(
                            out=oo[pair][
                                :, 512 * lj + cu.start : 512 * lj + cu.stop
                            ],
                            in_=ot_sb[:, cu],
                        )

                # groups: (lj, gi) — bank lj accumulates s-tiles 0..4lj+3 in
                # groups of 4; gi == lj is the diagonal (partial) group.
                # Software-pipelined: PE stays one group ahead of AV.
                groups = [(lj, gi) for lj in range(NB) for gi in range(lj + 1)]
                pending = []
                for gk, (lj, gi) in enumerate(groups):
                    if lj not in ot_banks:
                        ot_banks[lj] = otp.tile(
                            [E + 1, 512], f32, tag="ot", name="ot"
                        )
                    a_grp, bases = emit_st_group(lj, gi)
                    pending.append((lj, gi, a_grp, bases))
                    lag = 1
                    while len(pending) > lag:
                        plj, pgi, pa, pb = pending.pop(0)
                        emit_av_group(plj, pgi, pa, pb)
                        if pgi == plj:  # last group of bank plj
                            epilogue(plj)
                for plj, pgi, pa, pb in pending:
                    emit_av_group(plj, pgi, pa, pb)
                    if pgi == plj:
                        epilogue(
                            plj,
                            sliced=(
                                pair == PAIRS_PER_CORE - 1 and plj == NB - 1
                            ),
                        )

    return nc


# ---------------------------------------------------------------------------
# Host-side sharding / unsharding
# ---------------------------------------------------------------------------

def _in_maps(queries, keys, values, tau, delta, st_dtype=bf16, av_dtype=bf16):
    np_st = mybir.dt.np(st_dtype)
    np_av = mybir.dt.np(av_dtype)
    NT = L // 128
    mask = np.triu(np.ones((128, 128), dtype=np.float32)).astype(np_av)
    maps = []
    for c in range(N_CORES):
        ps = [2 * c, 2 * c + 1]
        b = ps[0] // H
        hs = [p % H for p in ps]
        qscale = np.float32(SCALE * tau[b, 0])
        qt = np.ascontiguousarray(
            np.stack([queries[b, :, h, :].T * qscale for h in hs])
        ).astype(np_st)
        # K^T packed even/odd s-tile blocks into the two partition halves
        kts = []
        for h in hs:
            ktf = keys[b, :, h, :].T.reshape(E, NT, 128)  # [E, tile, col]
            kts.append(
                np.concatenate(
                    [
                        ktf[:, 0::2, :].reshape(E, L // 2),
                        ktf[:, 1::2, :].reshape(E, L // 2),
                    ],
                    axis=0,
                )
            )
        kt = np.ascontiguousarray(np.stack(kts)).astype(np_st)
        # V augmented with the delta fold: cols 0..63 = V * exp(delta'),
        # col 64 = exp(delta') (denominator), col 65 pad. Pre-tiled to
        # [128, NT, 66] (l = t*128 + p) so the DMA is contiguous.
        expd = np.exp(SCALE * delta[b]).astype(np.float32)  # [L]
        vv = np.zeros((PAIRS_PER_CORE, L, E + 2), dtype=np.float32)
        for i, h in enumerate(hs):
            vv[i, :, 0:E] = values[b, :, h, :] * expd[:, None]
            vv[i, :, E] = expd
        vv = vv.reshape(PAIRS_PER_CORE, NT, 128, E + 2).transpose(0, 2, 1, 3)
        vv = np.ascontiguousarray(vv).astype(np_av)
        maps.append({"qt": qt, "kt": kt, "vv": vv, "mask": mask})
    return maps


_CACHED = {}


def run(queries, keys, values, tau, delta, trace=False, st_dtype=bf16,
        av_dtype=bf16):
    key = (str(st_dtype), str(av_dtype))
    if key not in _CACHED:
        _CACHED[key] = build_program(st_dtype, av_dtype)
    nc = _CACHED[key]
    in_maps = _in_maps(
        np.asarray(queries),
        np.asarray(keys),
        np.asarray(values),
        np.asarray(tau),
        np.asarray(delta),
        st_dtype=st_dtype,
        av_dtype=av_dtype,
    )
    res = run_bass_kernel_spmd(
        nc, in_maps, core_ids=list(range(N_CORES)), trace=trace
    )
    out = np.empty((B, L, H, E), dtype=np.float32)
    for c in range(N_CORES):
        o = res.results[c]["oo"]  # [PAIRS, E+1, L] raw accumulators
        for i, p in enumerate([2 * c, 2 * c + 1]):
            out[p // H, :, p % H, :] = (o[i][0:E] / o[i][E]).T
    return out, res


def kernel(queries, keys, values, tau, delta):
    out, _ = run(queries, keys, values, tau, delta, trace=False)
    return out


# revision 26
# speedup vs baseline: 1.1472x; 1.1472x over previous
"""De-stationary causal attention (B=2, L=S=2048, H=8, E=64) on 8 TRN2 cores.

Sharding: the 16 (batch, head) pairs are distributed 2-per-core (cores 0-3
get batch 0, heads 0..7; cores 4-7 get batch 1). Each core runs the same
Bass program on its two pairs.

Math: logits = (Q K^T) * (tau/sqrt(E)) + delta/sqrt(E), causal softmax, A V.
Host-side folds: Q is pre-scaled by tau/sqrt(E); exp(delta/sqrt(E)) is folded
into V (and into the appended denominator column), because
softmax(x + d)_s = exp(x_s) e^{d_s} / sum_j exp(x_j) e^{d_j}.
So the device only computes exp(q'k) with no bias.

Device structure per (b,h) pair, scores kept TRANSPOSED (s on partitions):
  bank-major over 4 output l-blocks of 512; for each bank, groups of 4
  s-tiles: ST[s,l] row-packed on the PE (two k=64 matmuls on partition halves
  run concurrently), exp on the ACT engine (diagonal groups repack their
  valid column ranges contiguously so 2 ACT calls cover a group instead of
  4), causal-mask multiply on the DVE, then AV row-packed into a [65,512]
  accumulator whose row 64 is the softmax denominator. The accumulator is
  copied to SBUF and DMA'd out RAW as [65, L] — the host does the
  denominator division and the [E,L] -> [L,E] transpose in numpy, which
  removes all PE transposes and DVE normalization from the device.

DMA: V arrives pre-tiled [128, NT, 66] and Q/K as whole [64, L] slabs
(one descriptor set each); the duplicate partition-half is an SBUF->SBUF
DMA on the GpSimd queue.
"""

import copy
import sys

import numpy as np

try:
    import concourse.bass as bass
except ImportError:  # pragma: no cover
    sys.path.insert(0, "/opt/trn_rl_repo")
    import concourse.bass as bass

import concourse.mybir as mybir
import concourse.tile as tile
from concourse.bass_utils import run_bass_kernel_spmd
from concourse.vector_clock import ScopedClock

B, L, H, E = 2, 2048, 8, 64
N_CORES = 8
PAIRS_PER_CORE = 2
SCALE = 1.0 / np.sqrt(np.float32(E))  # 0.125

f32 = mybir.dt.float32
f32r = mybir.dt.float32r
bf16 = mybir.dt.bfloat16

# ---------------------------------------------------------------------------
# Walrus in this toolchain rejects >1 sync-wait per instruction. Split extra
# waits onto NoOps committed just before the instruction on the same engine.
# ---------------------------------------------------------------------------
_NOP_TEMPLATE = {}


def _make_nop(engine, name):
    if engine not in _NOP_TEMPLATE:
        tmp = bass.Bass()
        _NOP_TEMPLATE[engine] = tmp.engines[engine].nop(nofuse=True).ins
    nop = copy.copy(_NOP_TEMPLATE[engine])
    nop.name = name
    nop.engine = engine
    nop.sync_info = None
    return nop


class SplitWaitTileContext(tile.TileContext):
    _ws_counter = 0

    def _split_waits(self, inst):
        si = inst.sync_info
        if si is None or not si.on_wait or len(si.on_wait) <= 1:
            return []
        if inst.engine == mybir.EngineType.Unassigned:
            return []
        waits = list(si.on_wait)
        inst.sync_info = mybir.SyncInfo(
            on_wait=[waits[0]], on_update=list(si.on_update or [])
        )
        nops = []
        for w in waits[1:]:
            SplitWaitTileContext._ws_counter += 1
            nop = _make_nop(inst.engine, f"I-ws{SplitWaitTileContext._ws_counter}")
            nop.sync_info = mybir.SyncInfo(on_wait=[w], on_update=[])
            nops.append(nop)
        return nops

    def _commit_instruction(self, inst, lazy_reg_writes=True):
        for nop in self._split_waits(inst):
            self._add_instruction(nop)
        super()._commit_instruction(inst, lazy_reg_writes)

    def _drain_and_barrier(self, tick_clock, wait_clock):
        nc = self.nc
        probe = nc.sync.nop(nofuse=True)
        wait_clock.add_sem_waits(
            probe.ins, ScopedClock({None: tick_clock.global_clock})
        )
        waits = list(probe.ins.sync_info.on_wait or []) if probe.ins.sync_info else []
        if len(waits) > 1:
            probe.ins.sync_info.on_wait = [waits[0]]
            handles = {h.num: h for h in self.sems.allocated().values()}
            for w in waits[1:]:
                nop = nc.sync.nop(nofuse=True)
                nop.wait_op(handles[w.id], w.wait_value, "sem-ge")
        nc.sync.drain()

        nc.all_engine_barrier()
        assert self.sems is not None
        popped = nc._tile_sem_poison_stack.pop()
        assert popped is self._sem_poison
        nc.clear_and_free_semaphores(list(self.sems.allocated().values()))
        nc.all_engine_barrier()


# ---------------------------------------------------------------------------
# Program builder (bank-major, fully row-packed)
# ---------------------------------------------------------------------------

# Diagonal-group column plan: chunk c covers local l-cols [128c, 512) of the
# bank (width 512-128c). The valid ranges are packed contiguously: tile0
# holds c0|c1|c3 at columns 0/512/896, tile1 holds c2 at column 0 (a_grp
# column 1024). One ACT call per tile instead of one per chunk.
DIAG_BASE = {0: 0, 1: 512, 2: 1024, 3: 896}


def build_program(st_dtype=bf16, av_dtype=bf16):
    nc = bass.Bass()
    Exp = mybir.ActivationFunctionType.Exp

    VW = E + 2  # v row: 64 values + denominator col + pad
    NT = L // 128  # 16 s-tiles / l-tiles
    NB = L // 512  # 4 OT banks

    qt = nc.declare_dram_parameter("qt", [PAIRS_PER_CORE, E, L], st_dtype, isOutput=False)
    # K^T packed even/odd: rows 0:64 hold the even 128-col s-tile blocks,
    # rows 64:128 the odd ones (the row-packed score matmuls only ever read
    # even s-tiles from the low partition half and odd from the high half,
    # so no duplicate K transfer is needed).
    kt = nc.declare_dram_parameter(
        "kt", [PAIRS_PER_CORE, 2 * E, L // 2], st_dtype, isOutput=False
    )
    vv = nc.declare_dram_parameter(
        "vv", [PAIRS_PER_CORE, 128, NT, VW], av_dtype, isOutput=False
    )
    mask = nc.declare_dram_parameter("mask", [128, 128], av_dtype, isOutput=False)
    oo = nc.declare_dram_parameter(
        "oo", [PAIRS_PER_CORE, E + 1, L], f32, isOutput=True
    )

    with SplitWaitTileContext(nc) as tc:
        with (
            tc.tile_pool(name="const", bufs=1) as constp,
            tc.tile_pool(name="qk", bufs=2) as qkp,
            tc.tile_pool(name="vp", bufs=2) as vp,
            tc.tile_pool(name="ap", bufs=4) as ap_pool,
            tc.tile_pool(name="ep", bufs=2) as ep,
            tc.tile_pool(name="st", bufs=3, space="PSUM") as stp,
            tc.tile_pool(name="otp", bufs=2, space="PSUM") as otp,
        ):
            mask_sb = constp.tile([128, 128], av_dtype, tag="mask")
            consts_loaded = []

            for pair in range(PAIRS_PER_CORE):
                # Q^T on partitions 0-63 with a copy on 64-127 (the stream
                # operand is needed on both halves); K^T arrives pre-packed
                # even/odd so one [128, L/2] transfer serves both halves.
                qt_sb = qkp.tile([2 * E, L], st_dtype, tag="qt")
                kt_sb = qkp.tile([2 * E, L // 2], st_dtype, tag="kt")
                # V slab [128, 16, 66]; col 64 carries exp(delta') for the
                # softmax denominator (host-folded), col 65 is padding.
                v_sb = vp.tile([128, NT, VW], av_dtype, tag="v")
                # pair 0 loads in small leading chunks so the first score
                # group starts as early as possible; pair 1 loads whole
                # while pair 0 computes. Queues: K + V on sync, Q low half
                # on scalar; the Q high-half duplicate is a cheap DVE
                # SBUF->SBUF copy (no extra HBM traffic).
                if pair == 0:
                    kchunks = [
                        slice(0, L // 8),
                        slice(L // 8, L // 4),
                        None,  # vv goes here
                        slice(L // 4, L // 2),
                    ]
                    qchunks = [
                        slice(0, L // 4),
                        slice(L // 4, L // 2),
                        slice(L // 2, L),
                    ]
                else:
                    kchunks = [slice(0, L // 2), None]
                    qchunks = [slice(0, L)]
                for cl in kchunks:
                    if cl is None:
                        nc.sync.dma_start(out=v_sb, in_=vv[pair])
                    else:
                        nc.sync.dma_start(out=kt_sb[:, cl], in_=kt[pair][:, cl])
                    if not consts_loaded:
                        # mask rides behind the first k chunk; first needed
                        # by group (0,0)'s diagonal multiply
                        nc.sync.dma_start(out=mask_sb, in_=mask[:])
                        consts_loaded.append(True)
                for cl in qchunks:
                    nc.scalar.dma_start(out=qt_sb[0:E, cl], in_=qt[pair][:, cl])
                    nc.vector.tensor_copy(
                        qt_sb[E : 2 * E, cl], qt_sb[0:E, cl]
                    )

                ot_banks = {}

                def emit_st_group(lj, gi):
                    """Returns (a_grp, bases) where bases[c] = (a_grp column
                    base, l-offset off) for chunk c."""
                    a_grp = ap_pool.tile(
                        [128, 4 * 512], av_dtype, tag="A", name="A"
                    )
                    if gi != lj:
                        for hb in range(2):
                            st = stp.tile(
                                [128, 1024], f32, tag="st", name="st"
                            )
                            for cc in range(2):
                                c = 2 * hb + cc
                                si = 4 * gi + c
                                half = (c % 2) * E
                                nc.tensor.matmul(
                                    st[:, 512 * cc : 512 * (cc + 1)],
                                    kt_sb[half : half + E, (si // 2) * 128 : (si // 2) * 128 + 128],
                                    qt_sb[half : half + E, 512 * lj : 512 * lj + 512],
                                    start=True,
                                    stop=True,
                                )
                            nc.scalar.activation(
                                out=a_grp[:, 1024 * hb : 1024 * (hb + 1)],
                                in_=st,
                                func=Exp,
                                scale=1.0,
                            )
                        return a_grp, [(512 * c, 0) for c in range(4)]
                    # Diagonal group: chunk c valid over l-cols [128c, 512).
                    # tile0 <- c0|c1|c3 packed at 0/512/896, tile1 <- c2 at 0.
                    st0 = stp.tile([128, 1024], f32, tag="st", name="st")
                    st1 = stp.tile([128, 1024], f32, tag="st", name="st")
                    for c in range(4):
                        si = 4 * gi + c
                        off = 128 * c
                        half = (c % 2) * E
                        dst = st1 if c == 2 else st0
                        base = 0 if c == 2 else DIAG_BASE[c]
                        nc.tensor.matmul(
                            dst[:, base : base + 512 - off],
                            kt_sb[half : half + E, (si // 2) * 128 : (si // 2) * 128 + 128],
                            qt_sb[half : half + E, 512 * lj + off : 512 * lj + 512],
                            start=True,
                            stop=True,
                        )
                    nc.scalar.activation(
                        out=a_grp[:, 0:1024], in_=st0, func=Exp, scale=1.0
                    )
                    nc.scalar.activation(
                        out=a_grp[:, 1024:1280],
                        in_=st1[:, 0:256],
                        func=Exp,
                        scale=1.0,
                    )
                    # causal mask on the leading 128 cols of each chunk
                    # (its diagonal block), split across DVE and gpsimd so
                    # the four multiplies run two-by-two in parallel.
                    for c in range(4):
                        base = DIAG_BASE[c]
                        eng = nc.vector if c % 2 == 0 else nc.gpsimd
                        eng.tensor_mul(
                            a_grp[:, base : base + 128],
                            a_grp[:, base : base + 128],
                            mask_sb,
                        )
                    return a_grp, [
                        (DIAG_BASE[c], 128 * c) for c in range(4)
                    ]

                def emit_av_group(lj, gi, a_grp, bases):
                    ot = ot_banks[lj]
                    for c in range(4):
                        si = 4 * gi + c
                        base, off = bases[c]
                        nc.tensor.matmul(
                            ot[:, off:512],
                            v_sb[:, si, 0 : E + 1],
                            a_grp[:, base : base + 512 - off],
                            start=(gi == 0 and c == 0),
                            stop=(gi == lj and c == 3),
                        )

                def epilogue(lj, sliced=False):
                    # Copy the raw accumulator (row 64 = denominator) to
                    # SBUF and ship it out; the host divides + transposes.
                    # The very last bank is sliced in half so its store
                    # overlaps the second copy.
                    ot = ot_banks.pop(lj)
                    ot_sb = ep.tile([E + 1, 512], f32, tag="ot_sb", name="ot_sb")
                    cuts = [slice(0, 256), slice(256, 512)] if sliced else [
                        slice(0, 512)
                    ]
                    for cu in cuts:
                        nc.vector.tensor_copy(ot_sb[:, cu], ot[:, cu])
                        nc.sync.dma_start(
                            out=oo[pair][
                                :, 512 * lj + cu.start : 512 * lj + cu.stop
                            ],
                            in_=ot_sb[:, cu],
                        )

                # groups: (lj, gi) — bank lj accumulates s-tiles 0..4lj+3 in
                # groups of 4; gi == lj is the diagonal (partial) group.
                # Software-pipelined: PE stays one group ahead of AV.
                groups = [(lj, gi) for lj in range(NB) for gi in range(lj + 1)]
                pending = []
                for gk, (lj, gi) in enumerate(groups):
                    if lj not in ot_banks:
                        ot_banks[lj] = otp.tile(
                            [E + 1, 512], f32, tag="ot", name="ot"
                        )
                    a_grp, bases = emit_st_group(lj, gi)
                    pending.append((lj, gi, a_grp, bases))
                    lag = 1
                    while len(pending) > lag:
                        plj, pgi, pa, pb = pending.pop(0)
                        emit_av_group(plj, pgi, pa, pb)
                        if pgi == plj:  # last group of bank plj
                            epilogue(plj)
                for plj, pgi, pa, pb in pending:
                    emit_av_group(plj, pgi, pa, pb)
                    if pgi == plj:
                        epilogue(
                            plj,
                            sliced=(
                                pair == PAIRS_PER_CORE - 1 and plj == NB - 1
                            ),
                        )

    return nc


# ---------------------------------------------------------------------------
# Host-side sharding / unsharding
# ---------------------------------------------------------------------------

def _in_maps(queries, keys, values, tau, delta, st_dtype=bf16, av_dtype=bf16):
    np_st = mybir.dt.np(st_dtype)
    np_av = mybir.dt.np(av_dtype)
    NT = L // 128
    mask = np.triu(np.ones((128, 128), dtype=np.float32)).astype(np_av)
    maps = []
    for c in range(N_CORES):
        ps = [2 * c, 2 * c + 1]
        b = ps[0] // H
        hs = [p % H for p in ps]
        qscale = np.float32(SCALE * tau[b, 0])
        qt = np.ascontiguousarray(
            np.stack([queries[b, :, h, :].T * qscale for h in hs])
        ).astype(np_st)
        # K^T packed even/odd s-tile blocks into the two partition halves
        kts = []
        for h in hs:
            ktf = keys[b, :, h, :].T.reshape(E, NT, 128)  # [E, tile, col]
            kts.append(
                np.concatenate(
                    [
                        ktf[:, 0::2, :].reshape(E, L // 2),
                        ktf[:, 1::2, :].reshape(E, L // 2),
                    ],
                    axis=0,
                )
            )
        kt = np.ascontiguousarray(np.stack(kts)).astype(np_st)
        # V augmented with the delta fold: cols 0..63 = V * exp(delta'),
        # col 64 = exp(delta') (denominator), col 65 pad. Pre-tiled to
        # [128, NT, 66] (l = t*128 + p) so the DMA is contiguous.
        expd = np.exp(SCALE * delta[b]).astype(np.float32)  # [L]
        vv = np.zeros((PAIRS_PER_CORE, L, E + 2), dtype=np.float32)
        for i, h in enumerate(hs):
            vv[i, :, 0:E] = values[b, :, h, :] * expd[:, None]
            vv[i, :, E] = expd
        vv = vv.reshape(PAIRS_PER_CORE, NT, 128, E + 2).transpose(0, 2, 1, 3)
        vv = np.ascontiguousarray(vv).astype(np_av)
        maps.append({"qt": qt, "kt": kt, "vv": vv, "mask": mask})
    return maps


_CACHED = {}


def run(queries, keys, values, tau, delta, trace=False, st_dtype=bf16,
        av_dtype=bf16):
    key = (str(st_dtype), str(av_dtype))
    if key not in _CACHED:
        _CACHED[key] = build_program(st_dtype, av_dtype)
    nc = _CACHED[key]
    in_maps = _in_maps(
        np.asarray(queries),
        np.asarray(keys),
        np.asarray(values),
        np.asarray(tau),
        np.asarray(delta),
        st_dtype=st_dtype,
        av_dtype=av_dtype,
    )
    res = run_bass_kernel_spmd(
        nc, in_maps, core_ids=list(range(N_CORES)), trace=trace
    )
    out = np.empty((B, L, H, E), dtype=np.float32)
    for c in range(N_CORES):
        o = res.results[c]["oo"]  # [PAIRS, E+1, L] raw accumulators
        for i, p in enumerate([2 * c, 2 * c + 1]):
            out[p // H, :, p % H, :] = (o[i][0:E] / o[i][E]).T
    return out, res


def kernel(queries, keys, values, tau, delta):
    out, _ = run(queries, keys, values, tau, delta, trace=False)
    return out
